# revision 9
# baseline (speedup 1.0000x reference)
"""Trainium2 Bass kernel for nn_Attention: LN -> QKV -> per-head attention
(with k/v layernorm) -> output projection.

Sharding: 8 cores = 4 batches x 2 head-groups (8 heads each).  Each core
computes its batch's QKV restricted to its heads, runs attention for its
8 heads, and produces a partial output projection (contraction over its
512 inner features).  The host sums the two partials per batch and adds
all bias terms.

v3 design notes:
  - All matmul operands bf16 (moving bf16 streams 1 col/cycle; fp32 does
    not).  Scores for the head pair run as two row-tiled matmuls
    (tile_position (0,0) / (64,0)) which execute concurrently on the PE
    quadrants.
  - exp split per key-tile: the ACT engine computes one head exactly,
    DVE computes the other with a bf16 Schraudolph approximation
    (int16(A*x+B) bitcast to bf16, ~2% rms).  Parity alternates per sk
    so each head gets 50% exact keys.
  - attention-out stationary is [v | 1] (65 cols): row 64 accumulates
    the softmax denominator for free.
  - reciprocal via exp(-ln(den)) on ACT (tables already loaded; avoids
    the very slow DVE reciprocal).  Ln reads the PSUM den row directly.
  - denominators broadcast across partitions via a DRAM bounce;
    normalize multiply runs on GpSimd.
"""

import math
import os
import sys

import numpy as np

for _p in ("/opt/trn_rl_repo", "/root/.axon_site/_ro/trn_rl_repo"):
    if os.path.isdir(_p) and _p not in sys.path:
        sys.path.append(_p)

import concourse.bass as bass
import concourse.mybir as mybir
import concourse.tile as tile
from concourse.bass_utils import run_bass_kernel_spmd

FP32 = mybir.dt.float32
FP32R = mybir.dt.float32r
BF16 = mybir.dt.bfloat16
I16 = mybir.dt.int16
AF = mybir.ActivationFunctionType
OP = mybir.AluOpType

B = 4            # batch
S = 2048         # sequence length
C = 1024         # model dim
HEADS = 16
D = 64           # head dim
HG = 8           # heads per core
F = HG * D       # per-core q/k/v feature width (512)
O = 1024         # output dim
P = 128
EPS = 1e-5
N_CORES = 8

S_TILES = S // P          # 16
C_TILES = C // P          # 8
SB = 4                    # seq blocks
SBW = S // SB             # 512 cols per seq block
PAIRS = HG // 2           # 4 head pairs per core
Q4 = 4                    # query blocks of 512
SCALE = D ** -0.5

# bf16 Schraudolph exp: e^x ~= bitcast_bf16(int16(A*x + B))
SCH_A = 128.0 / math.log(2.0)
SCH_B = 16250.5


def _bcast_ap(ap_1d, parts):
    """[n] DRAM AP -> [parts, n] with 0-step partition broadcast."""
    return bass.AP(tensor=ap_1d.tensor, offset=ap_1d.offset,
                   ap=[[0, parts]] + [list(x) for x in ap_1d.ap])


def _free_bcast(ap2d, n):
    """[p, m] AP -> [p, m, n] broadcasting each element n times along free."""
    return bass.AP(tensor=ap2d.tensor, offset=ap2d.offset,
                   ap=[list(x) for x in ap2d.ap] + [[0, n]])


def split_waits(nc, max_other=1):
    """walrus here rejects >1 sync-wait on TPB_CTRL (Drain) and may reject
    many on others; hoist extra waits onto preceding single-wait NoOps."""
    for f in nc.m.functions:
        for bb in f.blocks:
            new_insts = []
            for inst in bb.instructions:
                si = inst.sync_info
                limit = 1 if isinstance(
                    inst, (mybir.InstDrain, mybir.InstEventSemaphore,
                           mybir.InstNoOp)) else max_other
                if si and si.on_wait and len(si.on_wait) > limit:
                    waits = list(si.on_wait)
                    keep, extra = waits[-limit:], waits[:-limit]
                    for j, w in enumerate(extra):
                        nop = mybir.InstNoOp(
                            name=f"{inst.name}_wsplit_{j}", ins=[], outs=[])
                        nop.engine = inst.engine
                        nop.sync_info = mybir.SyncInfo(on_wait=[w], on_update=[])
                        new_insts.append(nop)
                    inst.sync_info = mybir.SyncInfo(
                        on_wait=keep, on_update=list(si.on_update))
                new_insts.append(inst)
            bb.instructions[:] = new_insts

    return nc


def build_nc(reps=None):
    from contextlib import ExitStack
    from concourse.masks import make_identity

    nc = bass.Bass()
    x_d = nc.declare_dram_parameter("x_s", [S, C], FP32, isOutput=False)
    wq_d = nc.declare_dram_parameter("wq", [C, F], BF16, isOutput=False)
    wkv_d = nc.declare_dram_parameter("wkv", [C, 2 * F], BF16, isOutput=False)
    wo_d = nc.declare_dram_parameter("wo", [F, O], BF16, isOutput=False)
    cq_d = nc.declare_dram_parameter("cq", [F], FP32, isOutput=False)
    ckv_d = nc.declare_dram_parameter("ckv", [2 * F], BF16, isOutput=False)
    out_d = nc.declare_dram_parameter("out_p", [S, O], FP32, isOutput=True)
    dbg = os.environ.get("K_DEBUG")
    if dbg:
        dbg_qT = nc.declare_dram_parameter("dbg_qT", [P, PAIRS, S], BF16,
                                           isOutput=True)
        dbg_kT = nc.declare_dram_parameter("dbg_kT", [P, PAIRS, S], BF16,
                                           isOutput=True)
        dbg_v = nc.declare_dram_parameter("dbg_v", [P, S_TILES, HG, D + 1],
                                          BF16, isOutput=True)
        dbg_att = nc.declare_dram_parameter("dbg_att", [P, PAIRS, S], BF16,
                                            isOutput=True)
        dbg_ldq = nc.declare_dram_parameter("dbg_ldq", [Q4, P, 2, SBW], FP32,
                                            isOutput=True)

    with tile.TileContext(nc) as tc, ExitStack() as ctx:
        singles = ctx.enter_context(tc.tile_pool(name="singles", bufs=1))
        acts = ctx.enter_context(tc.tile_pool(name="acts", bufs=1))

        # ---- persistent SBUF state (loaded once, outside the rep loop) ----
        ident_b = singles.tile([P, P], BF16)
        make_identity(nc, ident_b)
        eps_sb = singles.tile([P, 1], FP32)
        nc.vector.memset(eps_sb, EPS)

        wq_sb = singles.tile([P, C_TILES, F], BF16)
        nc.sync.dma_start(out=wq_sb, in_=wq_d.rearrange("(i p) f -> p i f", p=P))
        wkv_sb = singles.tile([P, C_TILES, 2 * F], BF16)
        nc.sync.dma_start(out=wkv_sb,
                          in_=wkv_d.rearrange("(i p) f -> p i f", p=P))
        wo_sb = singles.tile([P, F // P, O], BF16)
        nc.sync.dma_start(out=wo_sb, in_=wo_d.rearrange("(i p) o -> p i o", p=P))
        cq_sb = singles.tile([P, F // P], FP32)
        nc.sync.dma_start(out=cq_sb, in_=cq_d.rearrange("(i p) -> p i", p=P))
        ckv_bc = singles.tile([P, 2 * F], BF16)
        nc.sync.dma_start(out=ckv_bc, in_=_bcast_ap(ckv_d[:], P))

        qT_sb = acts.tile([P, PAIRS, S], BF16)
        kT_sb = acts.tile([P, S_TILES, PAIRS, P], BF16)
        v_sb = acts.tile([P, S_TILES, HG, D + 1], BF16)
        nc.vector.memset(v_sb[:, :, :, D], 1.0)
        attnT = acts.tile([P, PAIRS, S], BF16)

        if reps:
            ctx.enter_context(tc.For_i(0, reps, 1))

        # =========== phase 1+2: LN(x), transposes, Q/K|V projections =======
        # software pipelined: LN+transpose of seq-block sb+1 is emitted
        # inside the KV loop of sb so ACT/DVE work hides behind PE matmuls.
        with tc.tile_pool(name="p12", bufs=2) as p12, \
             tc.tile_pool(name="p12s", bufs=4) as p12s, \
             tc.tile_pool(name="ps12", bufs=1, space="PSUM") as ps12:
            xnT_blocks = [None] * SB

            def emit_ln(sb, t):
                """LN + transpose of seq tile t of block sb -> xnT slice."""
                if xnT_blocks[sb] is None:
                    xnT_blocks[sb] = p12.tile([P, C_TILES, SBW], BF16,
                                              tag="xnT", bufs=2, name="xnT")
                xnT = xnT_blocks[sb]
                row0 = sb * SBW + t * P
                x_t = p12.tile([P, C], FP32, tag="x", bufs=5, name="x_t")
                nc.sync.dma_start(out=x_t, in_=x_d[row0:row0 + P, :])
                sq_scr = p12.tile([P, C], FP32, tag="sqscr", bufs=3,
                                  name="sq_scr")
                xsq = p12s.tile([P, 1], FP32, tag="st1", name="xsq")
                nc.scalar.activation(sq_scr, x_t, AF.Square, accum_out=xsq)
                xsum = p12s.tile([P, 1], FP32, tag="st2", name="xsum")
                nc.vector.tensor_reduce(xsum, x_t,
                                        mybir.AxisListType.X, OP.add)
                mu = p12s.tile([P, 1], FP32, tag="st3", name="mu")
                nc.vector.tensor_scalar(mu, xsum, 1.0 / C, None, OP.mult)
                xsqm = p12s.tile([P, 1], FP32, tag="st4", name="xsqm")
                nc.vector.tensor_scalar(xsqm, xsq, 1.0 / C, None, OP.mult)
                musq = p12s.tile([P, 1], FP32, tag="st5", name="musq")
                nc.vector.tensor_mul(musq, mu, mu)
                var = p12s.tile([P, 1], FP32, tag="st6", name="var")
                nc.vector.tensor_sub(var, xsqm, musq)
                lnv = p12s.tile([P, 1], FP32, tag="st7", name="lnv")
                nc.scalar.activation(lnv, var, AF.Ln, bias=eps_sb)
                rstd = p12s.tile([P, 1], FP32, tag="st8", name="rstd")
                nc.scalar.activation(rstd, lnv, AF.Exp, scale=-0.5)
                nmr = p12s.tile([P, 1], FP32, tag="st9", name="nmr")
                nc.vector.tensor_scalar(nmr, mu, rstd, -1.0,
                                        OP.mult, OP.mult)
                xn_t = p12.tile([P, C], BF16, tag="xn", bufs=3, name="xn_t")
                nc.vector.tensor_scalar(xn_t, x_t, rstd, nmr,
                                        OP.mult, OP.add)
                for half in range(2):
                    tp = ps12.tile([P, 4, P], BF16, tag="tp", bufs=2,
                                   name="tp")
                    for cj in range(4):
                        ci = 4 * half + cj
                        nc.tensor.transpose(
                            tp[:, cj, :], xn_t[:, ci * P:(ci + 1) * P],
                            ident_b)
                    dst = xnT[:, 4 * half:4 * half + 4, t * P:(t + 1) * P]
                    if half == 0:
                        nc.vector.tensor_copy(dst, tp)
                    else:
                        nc.scalar.copy(dst, tp)

            for t in range(SBW // P):
                emit_ln(0, t)
            for sb in range(SB):
                xnT = xnT_blocks[sb]
                if sb + 1 < SB:
                    emit_ln(sb + 1, 0)
                # ---- Q projection (transposed out): qT = wq.T @ xnT ----
                for fp2 in range(2):
                    psq = ps12.tile([P, 2, SBW], FP32, tag="psq", bufs=1)
                    for ci in range(C_TILES):
                        for fj in range(2):
                            fi = 2 * fp2 + fj
                            nc.tensor.matmul(
                                psq[:, fj, :],
                                wq_sb[:, ci, fi * P:(fi + 1) * P],
                                xnT[:, ci, :],
                                start=(ci == 0), stop=(ci == C_TILES - 1))
                    for fj in range(2):
                        fi = 2 * fp2 + fj
                        nc.vector.tensor_scalar(
                            qT_sb[:, fi, sb * SBW:(sb + 1) * SBW],
                            psq[:, fj, :], cq_sb[:, fi:fi + 1], None, OP.add)

                # ---- K|V projection (natural out), stats, k transpose ----
                for st in range(SBW // P):
                    gst = sb * (SBW // P) + st
                    pskv = ps12.tile([P, 2 * F], FP32, tag="pskv", bufs=2)
                    for ci in range(C_TILES):
                        for half in range(2):
                            nc.tensor.matmul(
                                pskv[:, half * F:(half + 1) * F],
                                xnT[:, ci, st * P:(st + 1) * P],
                                wkv_sb[:, ci, half * F:(half + 1) * F],
                                start=(ci == 0), stop=(ci == C_TILES - 1))
                    # kvn = pskv + ckv (both k and v heads in one op)
                    kvn = p12.tile([P, 2 * HG, D], BF16, tag="kvn", bufs=3)
                    nc.vector.tensor_add(
                        kvn, pskv.rearrange("p (h d) -> p h d", d=D),
                        ckv_bc.rearrange("p (h d) -> p h d", d=D))
                    kvsq = p12.tile([P, 2 * HG, D], FP32, tag="kvsq", bufs=2)
                    nc.scalar.activation(kvsq, kvn, AF.Square)
                    kvvar = p12s.tile([P, 2 * HG], FP32, tag="kvvar")
                    nc.vector.tensor_reduce(kvvar, kvsq,
                                            mybir.AxisListType.X, OP.add)
                    kvlnv = p12s.tile([P, 2 * HG], FP32, tag="kvlnv")
                    nc.scalar.activation(kvlnv, kvvar, AF.Ln,
                                         bias=eps_sb, scale=1.0 / D)
                    kvrstd = p12s.tile([P, 2 * HG], FP32, tag="kvrstd")
                    nc.scalar.activation(kvrstd, kvlnv, AF.Exp, scale=-0.5)
                    khat = p12.tile([P, HG, D], BF16, tag="khat", bufs=3)
                    nc.gpsimd.tensor_mul(khat, kvn[:, 0:HG, :],
                                         _free_bcast(kvrstd[:, 0:HG], D))
                    nc.gpsimd.tensor_mul(v_sb[:, gst, :, 0:D],
                                         kvn[:, HG:2 * HG, :],
                                         _free_bcast(kvrstd[:, HG:2 * HG], D))
                    nc.sync.dma_start_transpose(
                        out=kT_sb[:, gst, :, :], in_=khat[:, :, :])
                    # pipelined LN of the next seq block (one tile ahead)
                    if sb + 1 < SB and st + 1 < SBW // P:
                        emit_ln(sb + 1, st + 1)

        if dbg:
            nc.sync.dma_start(out=dbg_qT[:, :, :], in_=qT_sb)
            nc.sync.dma_start(out=dbg_kT[:, :, :], in_=kT_sb)
            nc.sync.dma_start(out=dbg_v[:, :, :, :], in_=v_sb)

        # =========== phase 3: attention ====================================
        with tc.tile_pool(name="p3", bufs=3) as p3, \
             tc.tile_pool(name="p3r", bufs=2) as p3r, \
             tc.tile_pool(name="p3d", bufs=2, space="DRAM") as p3d, \
             tc.tile_pool(name="ps3", bufs=1, space="PSUM") as ps3:

            def emit_scores(q4, pj, sk):
                ps = ps3.tile([P, 2, SBW], FP32, tag="ps", bufs=2)
                qs = q4 * SBW
                par = sk % 2
                for h in (1 - par, par):   # approx-exp head's scores first
                    nc.tensor.matmul(ps[:, h, :],
                                     kT_sb[h * D:(h + 1) * D, sk, pj, :],
                                     qT_sb[h * D:(h + 1) * D, pj,
                                           qs:qs + SBW],
                                     tile_position=(h * D, 0))
                return ps

            stages = [(q4, pj, sk) for q4 in range(Q4)
                      for pj in range(PAIRS) for sk in range(S_TILES)]
            # three-deep pipeline: at stage i we emit scores(i+1), exp(i),
            # and attention-out(i-1).  PSUM: ps bufs=2 (4 banks) + poA/poB
            # bufs=2 (4 banks) = all 8 banks.
            po_tiles = {}      # pj-pair index of stage -> [poA, poB]
            e_tiles = {}
            ldq_tiles = {}

            def emit_exp(i):
                q4, pj, sk = stages[i]
                ps = ps_tiles.pop(i)
                par = sk % 2
                e = p3.tile([P, 2, SBW], BF16, tag="e", bufs=4, name="e")
                nc.vector.tensor_scalar(e[:, 1 - par, :].bitcast(I16),
                                        ps[:, 1 - par, :], SCH_A, SCH_B,
                                        OP.mult, OP.add)
                nc.scalar.activation(e[:, par, :], ps[:, par, :], AF.Exp)
                e_tiles[i] = e

            def emit_ao(i):
                q4, pj, sk = stages[i]
                qs = q4 * SBW
                if sk == 0:
                    po_tiles[i // S_TILES] = [
                        ps3.tile([D + 1, SBW], FP32, tag="poA", bufs=2,
                                 name="poA"),
                        ps3.tile([D + 1, SBW], FP32, tag="poB", bufs=2,
                                 name="poB")]
                po = po_tiles[i // S_TILES]
                e = e_tiles.pop(i)
                par = sk % 2
                for h in (par, 1 - par):   # exact-exp head first
                    nc.tensor.matmul(po[h], v_sb[:, sk, 2 * pj + h, :],
                                     e[:, h, :], start=(sk == 0),
                                     stop=(sk == S_TILES - 1))
                if sk == S_TILES - 1:
                    if pj == 0:
                        ldq_tiles[q4] = p3r.tile([P, 2, SBW], FP32,
                                                 tag="ldq", bufs=2,
                                                 name="ldq")
                    ldq = ldq_tiles[q4]
                    nc.scalar.copy(attnT[0:D, pj, qs:qs + SBW], po[0][0:D, :])
                    nc.vector.tensor_copy(attnT[D:P, pj, qs:qs + SBW],
                                          po[1][0:D, :])
                    r0 = 32 * pj
                    nc.scalar.activation(ldq[r0:r0 + 1, 0, :],
                                         po[0][D:D + 1, :], AF.Ln)
                    nc.scalar.activation(ldq[r0:r0 + 1, 1, :],
                                         po[1][D:D + 1, :], AF.Ln)
                    if pj == PAIRS - 1:
                        emit_q4_tail(q4)

            def emit_q4_tail(q4):
                qs = q4 * SBW
                ldq = ldq_tiles.pop(q4)
                if dbg:
                    nc.sync.dma_start(out=dbg_ldq[q4, :, :, :], in_=ldq)
                rec = p3r.tile([P, 2, SBW], BF16, tag="rec", bufs=2)
                nc.scalar.activation(rec, ldq, AF.Exp, scale=-1.0)
                for pjn in range(PAIRS):
                    rn = 32 * pjn
                    rdrp = p3d.tile([2, SBW], BF16, tag="rdr", bufs=8)
                    nc.sync.dma_start(out=rdrp, in_=rec[rn:rn + 1, :, :])
                    rbc2 = p3r.tile([P, SBW], BF16, tag="rbc2", bufs=4)
                    nc.sync.dma_start(
                        out=rbc2[0:D, :], in_=_bcast_ap(rdrp[0, :], D))
                    nc.sync.dma_start(
                        out=rbc2[D:P, :], in_=_bcast_ap(rdrp[1, :], D))
                    nc.gpsimd.tensor_mul(attnT[:, pjn, qs:qs + SBW],
                                         attnT[:, pjn, qs:qs + SBW],
                                         rbc2)

            ps_tiles = {0: emit_scores(*stages[0])}
            for i in range(len(stages)):
                if i + 1 < len(stages):
                    ps_tiles[i + 1] = emit_scores(*stages[i + 1])
                emit_exp(i)
                if i >= 2:
                    emit_ao(i - 2)
            emit_ao(len(stages) - 2)
            emit_ao(len(stages) - 1)

        if dbg:
            nc.sync.dma_start(out=dbg_att[:, :, :], in_=attnT)

        # =========== phase 4: output projection ============================
        with tc.tile_pool(name="p4", bufs=2) as p4, \
             tc.tile_pool(name="ps4", bufs=1, space="PSUM") as ps4:
            for st in range(S_TILES):
                pp = ps4.tile([P, O], FP32, tag="pp", bufs=2)
                for ii in range(F // P):
                    for half in range(2):
                        nc.tensor.matmul(
                            pp[:, half * F:(half + 1) * F],
                            attnT[:, ii, st * P:(st + 1) * P],
                            wo_sb[:, ii, half * F:(half + 1) * F],
                            start=(ii == 0), stop=(ii == F // P - 1))
                o_t = p4.tile([P, O], FP32, tag="ot", bufs=3)
                if st % 2 == 0:
                    nc.vector.tensor_copy(o_t, pp)
                else:
                    nc.scalar.copy(o_t, pp)
                nc.sync.dma_start(out=out_d[st * P:(st + 1) * P, :], in_=o_t)

    return nc


_NC_CACHE = None


def _get_nc():
    global _NC_CACHE
    if _NC_CACHE is None:
        nc = build_nc()
        split_waits(nc)
        _NC_CACHE = nc
    return _NC_CACHE


def _bf16(a):
    import ml_dtypes
    return np.ascontiguousarray(a.astype(ml_dtypes.bfloat16))


def prep_core_inputs(x, norm_g, norm_b, w_qkv, normk_g, normk_b,
                     normv_g, normv_b, w_out, b_out):
    """Host-side fold + shard.  Returns (in_maps, host_bias[core] (O,))."""
    x = np.asarray(x, np.float32)
    norm_g = np.asarray(norm_g, np.float32)
    norm_b = np.asarray(norm_b, np.float32)
    w_qkv = np.asarray(w_qkv, np.float32)
    normk_g = np.asarray(normk_g, np.float32)
    normv_g = np.asarray(normv_g, np.float32)
    normv_b = np.asarray(normv_b, np.float32)
    w_out = np.asarray(w_out, np.float32)

    INNER = HEADS * D
    wq_all, wk_all, wv_all = (w_qkv[:, 0:INNER], w_qkv[:, INNER:2 * INNER],
                              w_qkv[:, 2 * INNER:3 * INNER])
    gk_t = np.tile(normk_g, HG)          # [512] per head-group tiling
    gv_full = np.tile(normv_g, HEADS)
    bv_full = np.tile(normv_b, HEADS)

    def center(w):
        """subtract per-head column mean: w [*, 8, 64] blocks."""
        w3 = w.reshape(w.shape[0], HG, D) if w.ndim == 2 else w.reshape(HG, D)
        w3 = w3 - w3.mean(axis=-1, keepdims=True)
        return w3.reshape(w.shape)

    in_maps, host_bias = [], []
    for core in range(N_CORES):
        b_idx, hg = divmod(core, 2)
        cols = slice(hg * F, (hg + 1) * F)
        wq = wq_all[:, cols]
        wk = wk_all[:, cols]
        wv = wv_all[:, cols]
        wo = w_out[cols, :]
        wq_f = (norm_g[:, None] * wq) * (gk_t[None, :] * SCALE)
        cq = (norm_b @ wq) * gk_t * SCALE
        wk_f = center(norm_g[:, None] * wk)
        ck = center(norm_b @ wk)
        wv_f = center(norm_g[:, None] * wv)
        cv = center(norm_b @ wv)
        wo_f = gv_full[cols][:, None] * wo
        host_bias.append(bv_full[cols] @ wo)
        in_maps.append({
            "x_s": np.ascontiguousarray(x[b_idx]),
            "wq": _bf16(wq_f),
            "wkv": _bf16(np.concatenate([wk_f, wv_f], axis=1)),
            "wo": _bf16(wo_f),
            "cq": np.ascontiguousarray(cq),
            "ckv": _bf16(np.concatenate([ck, cv])),
        })
    return in_maps, host_bias


def kernel(**inputs):
    nc = _get_nc()
    in_maps, host_bias = prep_core_inputs(**inputs)
    res = run_bass_kernel_spmd(nc, in_maps, list(range(N_CORES)))
    b_out = np.asarray(inputs["b_out"], np.float32)
    out = np.empty((B, S, O), np.float32)
    for b_idx in range(B):
        out[b_idx] = (res.results[2 * b_idx]["out_p"]
                      + res.results[2 * b_idx + 1]["out_p"]
                      + host_bias[2 * b_idx] + host_bias[2 * b_idx + 1]
                      + b_out)
    return out


if __name__ == "__main__":
    nc = build_nc()
    n = sum(len(bb.instructions) for f in nc.m.functions for bb in f.blocks)
    print("built ok,", n, "instructions")


# revision 10
# speedup vs baseline: 1.4181x; 1.4181x over previous
"""Trainium2 Bass kernel for nn_Attention: LN -> QKV -> per-head attention
(with k/v layernorm) -> output projection.

Sharding: 8 cores = 4 batches x 2 head-groups (8 heads each).  Each core
computes its batch's QKV restricted to its heads, runs attention for its
8 heads, and produces a partial output projection (contraction over its
512 inner features).  The host sums the two partials per batch and adds
all bias terms.

v3 design notes:
  - All matmul operands bf16 (moving bf16 streams 1 col/cycle; fp32 does
    not).  Scores for the head pair run as two row-tiled matmuls
    (tile_position (0,0) / (64,0)) which execute concurrently on the PE
    quadrants.
  - exp split per key-tile: the ACT engine computes one head exactly,
    DVE computes the other with a bf16 Schraudolph approximation
    (int16(A*x+B) bitcast to bf16, ~2% rms).  Parity alternates per sk
    so each head gets 50% exact keys.
  - attention-out stationary is [v | 1] (65 cols): row 64 accumulates
    the softmax denominator for free.
  - reciprocal via exp(-ln(den)) on ACT (tables already loaded; avoids
    the very slow DVE reciprocal).  Ln reads the PSUM den row directly.
  - denominators broadcast across partitions via a DRAM bounce;
    normalize multiply runs on GpSimd.
"""

import math
import os
import sys

import numpy as np

for _p in ("/opt/trn_rl_repo", "/root/.axon_site/_ro/trn_rl_repo"):
    if os.path.isdir(_p) and _p not in sys.path:
        sys.path.append(_p)

import concourse.bass as bass
import concourse.mybir as mybir
import concourse.tile as tile
from concourse.bass_utils import run_bass_kernel_spmd

FP32 = mybir.dt.float32
FP32R = mybir.dt.float32r
BF16 = mybir.dt.bfloat16
I16 = mybir.dt.int16
AF = mybir.ActivationFunctionType
OP = mybir.AluOpType

B = 4            # batch
S = 2048         # sequence length
C = 1024         # model dim
HEADS = 16
D = 64           # head dim
HG = 8           # heads per core
F = HG * D       # per-core q/k/v feature width (512)
O = 1024         # output dim
P = 128
EPS = 1e-5
N_CORES = 8

S_TILES = S // P          # 16
C_TILES = C // P          # 8
SB = 4                    # seq blocks
SBW = S // SB             # 512 cols per seq block
PAIRS = HG // 2           # 4 head pairs per core
Q4 = 4                    # query blocks of 512
SCALE = D ** -0.5

# bf16 Schraudolph exp: e^x ~= bitcast_bf16(int16(A*x + B))
SCH_A = 128.0 / math.log(2.0)
SCH_B = 16250.5


def _bcast_ap(ap_1d, parts):
    """[n] DRAM AP -> [parts, n] with 0-step partition broadcast."""
    return bass.AP(tensor=ap_1d.tensor, offset=ap_1d.offset,
                   ap=[[0, parts]] + [list(x) for x in ap_1d.ap])


def _free_bcast(ap2d, n):
    """[p, m] AP -> [p, m, n] broadcasting each element n times along free."""
    return bass.AP(tensor=ap2d.tensor, offset=ap2d.offset,
                   ap=[list(x) for x in ap2d.ap] + [[0, n]])


def split_waits(nc, max_other=1):
    """walrus here rejects >1 sync-wait on TPB_CTRL (Drain) and may reject
    many on others; hoist extra waits onto preceding single-wait NoOps."""
    for f in nc.m.functions:
        for bb in f.blocks:
            new_insts = []
            for inst in bb.instructions:
                si = inst.sync_info
                limit = 1 if isinstance(
                    inst, (mybir.InstDrain, mybir.InstEventSemaphore,
                           mybir.InstNoOp)) else max_other
                if si and si.on_wait and len(si.on_wait) > limit:
                    waits = list(si.on_wait)
                    keep, extra = waits[-limit:], waits[:-limit]
                    for j, w in enumerate(extra):
                        nop = mybir.InstNoOp(
                            name=f"{inst.name}_wsplit_{j}", ins=[], outs=[])
                        nop.engine = inst.engine
                        nop.sync_info = mybir.SyncInfo(on_wait=[w], on_update=[])
                        new_insts.append(nop)
                    inst.sync_info = mybir.SyncInfo(
                        on_wait=keep, on_update=list(si.on_update))
                new_insts.append(inst)
            bb.instructions[:] = new_insts

    return nc


def build_nc(reps=None):
    from contextlib import ExitStack
    from concourse.masks import make_identity

    nc = bass.Bass()
    x_d = nc.declare_dram_parameter("x_s", [S, C], FP32, isOutput=False)
    wq_d = nc.declare_dram_parameter("wq", [C, F], BF16, isOutput=False)
    wkv_d = nc.declare_dram_parameter("wkv", [C, 2 * F], BF16, isOutput=False)
    wo_d = nc.declare_dram_parameter("wo", [F, O], BF16, isOutput=False)
    cq_d = nc.declare_dram_parameter("cq", [F], FP32, isOutput=False)
    ckv_d = nc.declare_dram_parameter("ckv", [2 * F], BF16, isOutput=False)
    out_d = nc.declare_dram_parameter("out_p", [S, O], FP32, isOutput=True)
    dbg = os.environ.get("K_DEBUG")
    if dbg:
        dbg_qT = nc.declare_dram_parameter("dbg_qT", [P, PAIRS, S], BF16,
                                           isOutput=True)
        dbg_kT = nc.declare_dram_parameter("dbg_kT", [P, PAIRS, S], BF16,
                                           isOutput=True)
        dbg_v = nc.declare_dram_parameter("dbg_v", [P, S_TILES, HG, D + 1],
                                          BF16, isOutput=True)
        dbg_att = nc.declare_dram_parameter("dbg_att", [P, PAIRS, S], BF16,
                                            isOutput=True)
        dbg_ldq = nc.declare_dram_parameter("dbg_ldq", [Q4, P, 2, SBW], FP32,
                                            isOutput=True)

    with tile.TileContext(nc) as tc, ExitStack() as ctx:
        singles = ctx.enter_context(tc.tile_pool(name="singles", bufs=1))
        acts = ctx.enter_context(tc.tile_pool(name="acts", bufs=1))

        # ---- persistent SBUF state (loaded once, outside the rep loop) ----
        ident_b = singles.tile([P, P], BF16)
        make_identity(nc, ident_b)
        eps_sb = singles.tile([P, 1], FP32)
        nc.vector.memset(eps_sb, EPS)

        wq_sb = singles.tile([P, C_TILES, F], BF16)
        nc.sync.dma_start(out=wq_sb, in_=wq_d.rearrange("(i p) f -> p i f", p=P))
        wkv_sb = singles.tile([P, C_TILES, 2 * F], BF16)
        nc.sync.dma_start(out=wkv_sb,
                          in_=wkv_d.rearrange("(i p) f -> p i f", p=P))
        wo_sb = singles.tile([P, F // P, O], BF16)
        nc.sync.dma_start(out=wo_sb, in_=wo_d.rearrange("(i p) o -> p i o", p=P))
        cq_sb = singles.tile([P, F // P], FP32)
        nc.sync.dma_start(out=cq_sb, in_=cq_d.rearrange("(i p) -> p i", p=P))
        ckv_bc = singles.tile([P, 2 * F], BF16)
        nc.sync.dma_start(out=ckv_bc, in_=_bcast_ap(ckv_d[:], P))

        qT_sb = acts.tile([P, PAIRS, S], BF16)
        kT_sb = acts.tile([P, PAIRS, S], BF16)
        v_sb = acts.tile([P, S_TILES, HG, D + 1], BF16)
        nc.vector.memset(v_sb[:, :, :, D], 1.0)
        attnT = acts.tile([P, PAIRS, S], BF16)

        if reps:
            ctx.enter_context(tc.For_i(0, reps, 1))

        # =========== phase 1+2: LN(x), transposes, Q/K|V projections =======
        # software pipelined: LN+transpose of seq-block sb+1 is emitted
        # inside the KV loop of sb so ACT/DVE work hides behind PE matmuls.
        with tc.tile_pool(name="p12", bufs=2) as p12, \
             tc.tile_pool(name="p12s", bufs=4) as p12s, \
             tc.tile_pool(name="ps12", bufs=1, space="PSUM") as ps12:
            xnT_blocks = [None] * SB

            def emit_ln(sb, t):
                """LN + transpose of seq tile t of block sb -> xnT slice."""
                if xnT_blocks[sb] is None:
                    xnT_blocks[sb] = p12.tile([P, C_TILES, SBW], BF16,
                                              tag="xnT", bufs=2, name="xnT")
                xnT = xnT_blocks[sb]
                row0 = sb * SBW + t * P
                x_t = p12.tile([P, C], FP32, tag="x", bufs=5, name="x_t")
                nc.sync.dma_start(out=x_t, in_=x_d[row0:row0 + P, :])
                sq_scr = p12.tile([P, C], FP32, tag="sqscr", bufs=3,
                                  name="sq_scr")
                xsq = p12s.tile([P, 1], FP32, tag="st1", name="xsq")
                nc.scalar.activation(sq_scr, x_t, AF.Square, accum_out=xsq)
                xsum = p12s.tile([P, 1], FP32, tag="st2", name="xsum")
                nc.vector.tensor_reduce(xsum, x_t,
                                        mybir.AxisListType.X, OP.add)
                mu = p12s.tile([P, 1], FP32, tag="st3", name="mu")
                nc.vector.tensor_scalar(mu, xsum, 1.0 / C, None, OP.mult)
                xsqm = p12s.tile([P, 1], FP32, tag="st4", name="xsqm")
                nc.vector.tensor_scalar(xsqm, xsq, 1.0 / C, None, OP.mult)
                musq = p12s.tile([P, 1], FP32, tag="st5", name="musq")
                nc.vector.tensor_mul(musq, mu, mu)
                var = p12s.tile([P, 1], FP32, tag="st6", name="var")
                nc.vector.tensor_sub(var, xsqm, musq)
                lnv = p12s.tile([P, 1], FP32, tag="st7", name="lnv")
                nc.scalar.activation(lnv, var, AF.Ln, bias=eps_sb)
                rstd = p12s.tile([P, 1], FP32, tag="st8", name="rstd")
                nc.scalar.activation(rstd, lnv, AF.Exp, scale=-0.5)
                nmr = p12s.tile([P, 1], FP32, tag="st9", name="nmr")
                nc.vector.tensor_scalar(nmr, mu, rstd, -1.0,
                                        OP.mult, OP.mult)
                xn_t = p12.tile([P, C], BF16, tag="xn", bufs=3, name="xn_t")
                nc.vector.tensor_scalar(xn_t, x_t, rstd, nmr,
                                        OP.mult, OP.add)
                for half in range(2):
                    tp = ps12.tile([P, 4, P], BF16, tag="tp", bufs=2,
                                   name="tp")
                    for cj in range(4):
                        ci = 4 * half + cj
                        nc.tensor.transpose(
                            tp[:, cj, :], xn_t[:, ci * P:(ci + 1) * P],
                            ident_b)
                    dst = xnT[:, 4 * half:4 * half + 4, t * P:(t + 1) * P]
                    if half == 0:
                        nc.vector.tensor_copy(dst, tp)
                    else:
                        nc.scalar.copy(dst, tp)

            for t in range(SBW // P):
                emit_ln(0, t)
            for sb in range(SB):
                xnT = xnT_blocks[sb]
                if sb + 1 < SB:
                    emit_ln(sb + 1, 0)
                # ---- Q projection (transposed out): qT = wq.T @ xnT ----
                for fp2 in range(2):
                    psq = ps12.tile([P, 2, SBW], FP32, tag="psq", bufs=1)
                    for ci in range(C_TILES):
                        for fj in range(2):
                            fi = 2 * fp2 + fj
                            nc.tensor.matmul(
                                psq[:, fj, :],
                                wq_sb[:, ci, fi * P:(fi + 1) * P],
                                xnT[:, ci, :],
                                start=(ci == 0), stop=(ci == C_TILES - 1))
                    for fj in range(2):
                        fi = 2 * fp2 + fj
                        nc.vector.tensor_scalar(
                            qT_sb[:, fi, sb * SBW:(sb + 1) * SBW],
                            psq[:, fj, :], cq_sb[:, fi:fi + 1], None, OP.add)

                # ---- K|V projection (natural out), stats, k transpose ----
                for st in range(SBW // P):
                    gst = sb * (SBW // P) + st
                    pskv = ps12.tile([P, 2 * F], FP32, tag="pskv", bufs=2)
                    for ci in range(C_TILES):
                        for half in range(2):
                            nc.tensor.matmul(
                                pskv[:, half * F:(half + 1) * F],
                                xnT[:, ci, st * P:(st + 1) * P],
                                wkv_sb[:, ci, half * F:(half + 1) * F],
                                start=(ci == 0), stop=(ci == C_TILES - 1))
                    # kvn = pskv + ckv (both k and v heads in one op)
                    kvn = p12.tile([P, 2 * HG, D], BF16, tag="kvn", bufs=3)
                    nc.vector.tensor_add(
                        kvn, pskv.rearrange("p (h d) -> p h d", d=D),
                        ckv_bc.rearrange("p (h d) -> p h d", d=D))
                    kvsq = p12.tile([P, 2 * HG, D], FP32, tag="kvsq", bufs=2)
                    nc.scalar.activation(kvsq, kvn, AF.Square)
                    kvvar = p12s.tile([P, 2 * HG], FP32, tag="kvvar")
                    nc.vector.tensor_reduce(kvvar, kvsq,
                                            mybir.AxisListType.X, OP.add)
                    kvlnv = p12s.tile([P, 2 * HG], FP32, tag="kvlnv")
                    nc.scalar.activation(kvlnv, kvvar, AF.Ln,
                                         bias=eps_sb, scale=1.0 / D)
                    kvrstd = p12s.tile([P, 2 * HG], FP32, tag="kvrstd")
                    nc.scalar.activation(kvrstd, kvlnv, AF.Exp, scale=-0.5)
                    khat = p12.tile([P, HG, D], BF16, tag="khat", bufs=3)
                    nc.gpsimd.tensor_mul(khat, kvn[:, 0:HG, :],
                                         _free_bcast(kvrstd[:, 0:HG], D))
                    nc.gpsimd.tensor_mul(v_sb[:, gst, :, 0:D],
                                         kvn[:, HG:2 * HG, :],
                                         _free_bcast(kvrstd[:, HG:2 * HG], D))
                    tpk = ps12.tile([P, 4, P], BF16, tag="tp", bufs=2)
                    for pj in range(PAIRS):
                        nc.tensor.transpose(
                            tpk[:, pj, :], khat[:, 2 * pj:2 * pj + 2, :],
                            ident_b)
                    dstk = kT_sb[:, :, gst * P:(gst + 1) * P]
                    if st % 2 == 0:
                        nc.vector.tensor_copy(dstk, tpk)
                    else:
                        nc.scalar.copy(dstk, tpk)
                    # pipelined LN of the next seq block (one tile ahead)
                    if sb + 1 < SB and st + 1 < SBW // P:
                        emit_ln(sb + 1, st + 1)

        if dbg:
            nc.sync.dma_start(out=dbg_qT[:, :, :], in_=qT_sb)
            nc.sync.dma_start(out=dbg_kT[:, :, :], in_=kT_sb)
            nc.sync.dma_start(out=dbg_v[:, :, :, :], in_=v_sb)

        # =========== phase 3: attention ====================================
        with tc.tile_pool(name="p3", bufs=3) as p3, \
             tc.tile_pool(name="p3r", bufs=2) as p3r, \
             tc.tile_pool(name="p3d", bufs=2, space="DRAM") as p3d, \
             tc.tile_pool(name="ps3", bufs=1, space="PSUM") as ps3:

            def emit_scores(q4, pj, sk):
                ps = ps3.tile([P, 2, SBW], FP32, tag="ps", bufs=2)
                qs, ks = q4 * SBW, sk * P
                par = sk % 2
                for h in (1 - par, par):   # approx-exp head's scores first
                    nc.tensor.matmul(ps[:, h, :],
                                     kT_sb[h * D:(h + 1) * D, pj, ks:ks + P],
                                     qT_sb[h * D:(h + 1) * D, pj,
                                           qs:qs + SBW],
                                     tile_position=(h * D, 0))
                return ps

            stages = [(q4, pj, sk) for q4 in range(Q4)
                      for pj in range(PAIRS) for sk in range(S_TILES)]
            # three-deep pipeline: at stage i we emit scores(i+1), exp(i),
            # and attention-out(i-1).  PSUM: ps bufs=2 (4 banks) + poA/poB
            # bufs=2 (4 banks) = all 8 banks.
            po_tiles = {}      # pj-pair index of stage -> [poA, poB]
            e_tiles = {}
            ldq_tiles = {}

            def emit_exp(i):
                q4, pj, sk = stages[i]
                ps = ps_tiles.pop(i)
                par = sk % 2
                e = p3.tile([P, 2, SBW], BF16, tag="e", bufs=4, name="e")
                nc.vector.tensor_scalar(e[:, 1 - par, :].bitcast(I16),
                                        ps[:, 1 - par, :], SCH_A, SCH_B,
                                        OP.mult, OP.add)
                nc.scalar.activation(e[:, par, :], ps[:, par, :], AF.Exp)
                e_tiles[i] = e

            def emit_ao(i):
                q4, pj, sk = stages[i]
                qs = q4 * SBW
                if sk == 0:
                    po_tiles[i // S_TILES] = [
                        ps3.tile([D + 1, SBW], FP32, tag="poA", bufs=2,
                                 name="poA"),
                        ps3.tile([D + 1, SBW], FP32, tag="poB", bufs=2,
                                 name="poB")]
                po = po_tiles[i // S_TILES]
                e = e_tiles.pop(i)
                par = sk % 2
                for h in (par, 1 - par):   # exact-exp head first
                    nc.tensor.matmul(po[h], v_sb[:, sk, 2 * pj + h, :],
                                     e[:, h, :], start=(sk == 0),
                                     stop=(sk == S_TILES - 1))
                if sk == S_TILES - 1:
                    if pj == 0:
                        ldq_tiles[q4] = p3r.tile([P, 2, SBW], FP32,
                                                 tag="ldq", bufs=2,
                                                 name="ldq")
                    ldq = ldq_tiles[q4]
                    nc.scalar.copy(attnT[0:D, pj, qs:qs + SBW], po[0][0:D, :])
                    nc.vector.tensor_copy(attnT[D:P, pj, qs:qs + SBW],
                                          po[1][0:D, :])
                    r0 = 32 * pj
                    nc.scalar.activation(ldq[r0:r0 + 1, 0, :],
                                         po[0][D:D + 1, :], AF.Ln)
                    nc.scalar.activation(ldq[r0:r0 + 1, 1, :],
                                         po[1][D:D + 1, :], AF.Ln)
                    if pj == PAIRS - 1:
                        emit_q4_tail(q4)

            def emit_q4_tail(q4):
                qs = q4 * SBW
                ldq = ldq_tiles.pop(q4)
                if dbg:
                    nc.sync.dma_start(out=dbg_ldq[q4, :, :, :], in_=ldq)
                rec = p3r.tile([P, 2, SBW], BF16, tag="rec", bufs=2)
                nc.scalar.activation(rec, ldq, AF.Exp, scale=-1.0)
                for pjn in range(PAIRS):
                    rn = 32 * pjn
                    rdrp = p3d.tile([2, SBW], BF16, tag="rdr", bufs=8)
                    nc.sync.dma_start(out=rdrp, in_=rec[rn:rn + 1, :, :])
                    rbc2 = p3r.tile([P, SBW], BF16, tag="rbc2", bufs=4)
                    nc.sync.dma_start(
                        out=rbc2[0:D, :], in_=_bcast_ap(rdrp[0, :], D))
                    nc.sync.dma_start(
                        out=rbc2[D:P, :], in_=_bcast_ap(rdrp[1, :], D))
                    nc.gpsimd.tensor_mul(attnT[:, pjn, qs:qs + SBW],
                                         attnT[:, pjn, qs:qs + SBW],
                                         rbc2)

            ps_tiles = {0: emit_scores(*stages[0])}
            for i in range(len(stages)):
                if i + 1 < len(stages):
                    ps_tiles[i + 1] = emit_scores(*stages[i + 1])
                emit_exp(i)
                if i >= 2:
                    emit_ao(i - 2)
            emit_ao(len(stages) - 2)
            emit_ao(len(stages) - 1)

        if dbg:
            nc.sync.dma_start(out=dbg_att[:, :, :], in_=attnT)

        # =========== phase 4: output projection ============================
        with tc.tile_pool(name="p4", bufs=2) as p4, \
             tc.tile_pool(name="ps4", bufs=1, space="PSUM") as ps4:
            for st in range(S_TILES):
                pp = ps4.tile([P, O], FP32, tag="pp", bufs=2)
                for ii in range(F // P):
                    for half in range(2):
                        nc.tensor.matmul(
                            pp[:, half * F:(half + 1) * F],
                            attnT[:, ii, st * P:(st + 1) * P],
                            wo_sb[:, ii, half * F:(half + 1) * F],
                            start=(ii == 0), stop=(ii == F // P - 1))
                o_t = p4.tile([P, O], FP32, tag="ot", bufs=3)
                if st % 2 == 0:
                    nc.vector.tensor_copy(o_t, pp)
                else:
                    nc.scalar.copy(o_t, pp)
                nc.sync.dma_start(out=out_d[st * P:(st + 1) * P, :], in_=o_t)

    return nc


_NC_CACHE = None


def _get_nc():
    global _NC_CACHE
    if _NC_CACHE is None:
        nc = build_nc()
        split_waits(nc)
        _NC_CACHE = nc
    return _NC_CACHE


def _bf16(a):
    import ml_dtypes
    return np.ascontiguousarray(a.astype(ml_dtypes.bfloat16))


def prep_core_inputs(x, norm_g, norm_b, w_qkv, normk_g, normk_b,
                     normv_g, normv_b, w_out, b_out):
    """Host-side fold + shard.  Returns (in_maps, host_bias[core] (O,))."""
    x = np.asarray(x, np.float32)
    norm_g = np.asarray(norm_g, np.float32)
    norm_b = np.asarray(norm_b, np.float32)
    w_qkv = np.asarray(w_qkv, np.float32)
    normk_g = np.asarray(normk_g, np.float32)
    normv_g = np.asarray(normv_g, np.float32)
    normv_b = np.asarray(normv_b, np.float32)
    w_out = np.asarray(w_out, np.float32)

    INNER = HEADS * D
    wq_all, wk_all, wv_all = (w_qkv[:, 0:INNER], w_qkv[:, INNER:2 * INNER],
                              w_qkv[:, 2 * INNER:3 * INNER])
    gk_t = np.tile(normk_g, HG)          # [512] per head-group tiling
    gv_full = np.tile(normv_g, HEADS)
    bv_full = np.tile(normv_b, HEADS)

    def center(w):
        """subtract per-head column mean: w [*, 8, 64] blocks."""
        w3 = w.reshape(w.shape[0], HG, D) if w.ndim == 2 else w.reshape(HG, D)
        w3 = w3 - w3.mean(axis=-1, keepdims=True)
        return w3.reshape(w.shape)

    in_maps, host_bias = [], []
    for core in range(N_CORES):
        b_idx, hg = divmod(core, 2)
        cols = slice(hg * F, (hg + 1) * F)
        wq = wq_all[:, cols]
        wk = wk_all[:, cols]
        wv = wv_all[:, cols]
        wo = w_out[cols, :]
        wq_f = (norm_g[:, None] * wq) * (gk_t[None, :] * SCALE)
        cq = (norm_b @ wq) * gk_t * SCALE
        wk_f = center(norm_g[:, None] * wk)
        ck = center(norm_b @ wk)
        wv_f = center(norm_g[:, None] * wv)
        cv = center(norm_b @ wv)
        wo_f = gv_full[cols][:, None] * wo
        host_bias.append(bv_full[cols] @ wo)
        in_maps.append({
            "x_s": np.ascontiguousarray(x[b_idx]),
            "wq": _bf16(wq_f),
            "wkv": _bf16(np.concatenate([wk_f, wv_f], axis=1)),
            "wo": _bf16(wo_f),
            "cq": np.ascontiguousarray(cq),
            "ckv": _bf16(np.concatenate([ck, cv])),
        })
    return in_maps, host_bias


def kernel(**inputs):
    nc = _get_nc()
    in_maps, host_bias = prep_core_inputs(**inputs)
    res = run_bass_kernel_spmd(nc, in_maps, list(range(N_CORES)))
    b_out = np.asarray(inputs["b_out"], np.float32)
    out = np.empty((B, S, O), np.float32)
    for b_idx in range(B):
        out[b_idx] = (res.results[2 * b_idx]["out_p"]
                      + res.results[2 * b_idx + 1]["out_p"]
                      + host_bias[2 * b_idx] + host_bias[2 * b_idx + 1]
                      + b_out)
    return out


if __name__ == "__main__":
    nc = build_nc()
    n = sum(len(bb.instructions) for f in nc.m.functions for bb in f.blocks)
    print("built ok,", n, "instructions")
